# revision 25
# baseline (speedup 1.0000x reference)
"""BBoxHead (nms_detection) Trainium2 Bass kernel, 8-core SPMD.

Strategy
--------
Data-parallel over the RoI axis: 2000 RoIs -> 250/core (padded to 256).
Activations are kept transposed as [feature, roi] on-chip so that:
  * every weight matrix loads from DRAM in its natural [K, M] layout
    (no transposes anywhere),
  * BatchNorm (training mode, batch stats over ALL 2000 RoIs) reduces along
    the free axis; the cross-core part of the stats is a tiny f32 AllReduce
    (sum and sum-of-squares per feature), split into two chunks per layer so
    the collective latency overlaps the surrounding matmuls,
  * the class/delta heads come out as [roi, class] directly (lhsT = x2n),
    so softmax also reduces along the free axis.
Matmul operands are fp16 (full PE rate, fp32 PSUM accumulation); everything
else (BN, softmax, biases, outputs) is fp32.  The conv biases b1/b2 cancel
exactly in training-mode BN and are skipped.  DMAs are emitted in first-use
order (rois/w1 slabs just-in-time, later-stage weights after mm1) so the PE
never waits behind loads it does not need yet.

Host-side prep reorders weights so each SBUF partition receives >=512B
contiguous chunks (two k-rows interleaved per partition).
"""

import numpy as np

import concourse.bass as bass
import concourse.mybir as mybir
import concourse.tile as tile
from concourse import bacc
from concourse import bass_utils

# ---------------------------------------------------------------- constants
N_CORES = 8
N = 2000                      # total RoIs
NLOC = N // N_CORES           # real RoIs per core (250)
NPAD = 256                    # padded RoI columns per core
K1 = 12544                    # 7*7*256 contraction of conv1
HID = 1024
NC = 81
ND = NC * 4                   # 324
NH = NC + ND                  # 405 (heads fused side by side)
EPS = 1e-3
KT1 = K1 // 128               # 98 k-tiles
KI = 7                        # k-tiles interleaved per partition chunk
KT7 = KT1 // KI               # 14 interleaved k-group tiles
# k-group tiles per w1/rois DMA slab (uniform; the HWDGE ring is FIFO so many
# small DMAs cost more than a faster start is worth)
SLABS = [2] * 7
assert sum(SLABS) == KT7
SLABOFF = [sum(SLABS[:i]) for i in range(len(SLABS))]
MT = HID // 128               # 8 feature tiles
SA1 = 7                       # BN1 stats chunk A covers m-tiles [0, SA1)
F16 = mybir.dt.float16
F32 = mybir.dt.float32

_CACHE = {}


def _build(iters=1, no_ar=False, sa1=SA1, sg2=4, rois_gp=False, warm=0):
    """Build the NEFF. iters>1 repeats the pipeline (timing harness);
    no_ar=True replaces the stats AllReduce with a local copy (timing only —
    wrong numerics, same structure). sa1/sg2 control the BN1/BN2 stats
    chunking (8 = single AllReduce per layer)."""
    nc = bacc.Bacc("TRN2", target_bir_lowering=False, debug=False,
                   enable_asserts=False, num_devices=N_CORES)

    # ------------------------------------------------ DRAM I/O declarations
    roisT_d = nc.dram_tensor("roisT", [KT7, 128, KI, NPAD], F16, kind="ExternalInput").ap()
    w1_d = nc.dram_tensor("w1t", [MT, KT7, 128, KI, 128], F16, kind="ExternalInput").ap()
    w2_d = nc.dram_tensor("w2t", [MT, 4, 128, 2, 128], F16, kind="ExternalInput").ap()
    wh_d = nc.dram_tensor("wht", [4, 128, 2, NH], F16, kind="ExternalInput").ap()
    bn_d = nc.dram_tensor("bnp", [128, 32], F32, kind="ExternalInput").ap()
    bias_d = nc.dram_tensor("biasb", [128, NH], F32, kind="ExternalInput").ap()

    logits_d = nc.dram_tensor("logits", [NLOC, NC], F32, kind="ExternalOutput").ap()
    probs_d = nc.dram_tensor("probs", [NLOC, NC], F32, kind="ExternalOutput").ap()
    deltas_d = nc.dram_tensor("deltas", [NLOC, ND], F32, kind="ExternalOutput").ap()

    with tile.TileContext(nc) as tc:
        with (
            tc.tile_pool(name="consts", bufs=1) as consts,
            tc.tile_pool(name="w1pool", bufs=4) as w1pool,
            tc.tile_pool(name="acts", bufs=1) as acts,
            tc.tile_pool(name="scr", bufs=2) as scr,
            tc.tile_pool(name="stats", bufs=1) as stats,
            tc.tile_pool(name="ppmm", bufs=6, space="PSUM") as ppmm,
            tc.tile_pool(name="pphd", bufs=1, space="PSUM") as pphd,
            tc.tile_pool(name="dram", bufs=1, space="DRAM") as dram,
        ):
          for _it in range(iters):
            # ---------------- tiny consts needed early (BN params, eps)
            bn_sb = consts.tile([128, 32], F32, tag="bn")
            nc.sync.dma_start(bn_sb[:], bn_d)
            eps_sb = consts.tile([128, 1], F32, tag="eps")
            nc.vector.memset(eps_sb[:], EPS)

            # --------------------------------------------------- helpers
            def bn_coeffs(gstats, m, gamma, beta, tag):
                """a = gamma*rsqrt(var+eps); c = beta - mu*a  (all [128, m]).
                gstats: [128, 2m] (sums | sumsqs) of x over the full batch."""
                mu = stats.tile([128, m], F32, tag=f"mu{tag}")
                ex2 = stats.tile([128, m], F32, tag=f"ex2{tag}")
                mu2 = stats.tile([128, m], F32, tag=f"mu2{tag}")
                var = stats.tile([128, m], F32, tag=f"var{tag}")
                std = stats.tile([128, m], F32, tag=f"std{tag}")
                rinv = stats.tile([128, m], F32, tag=f"rinv{tag}")
                a = stats.tile([128, m], F32, tag=f"a{tag}")
                ac = stats.tile([128, m], F32, tag=f"ac{tag}")
                c = stats.tile([128, m], F32, tag=f"c{tag}")
                nc.vector.tensor_scalar_mul(mu[:], gstats[:, 0:m], 1.0 / N)
                nc.vector.tensor_scalar_mul(ex2[:], gstats[:, m:2 * m], 1.0 / N)
                nc.vector.tensor_mul(mu2[:], mu[:], mu[:])
                nc.vector.tensor_sub(var[:], ex2[:], mu2[:])
                nc.scalar.activation(std[:], var[:], mybir.ActivationFunctionType.Sqrt,
                                     bias=eps_sb[:, 0:1])
                nc.vector.reciprocal(rinv[:], std[:])
                nc.vector.tensor_mul(a[:], gamma, rinv[:])
                nc.vector.tensor_mul(ac[:], mu[:], a[:])
                nc.vector.tensor_sub(c[:], beta, ac[:])
                return a, c

            def evict_with_stats(psum, x_sb, st, col, m=MT):
                """psum [128,NPAD] f32 -> x_sb; sum/sumsq of first NLOC cols
                into st columns col and m+col."""
                nc.vector.reduce_sum(st[:, col:col + 1], psum[:, :NLOC],
                                     axis=mybir.AxisListType.X)
                sq = scr.tile([128, NLOC], F32, tag="sq")
                nc.scalar.activation(sq[:], psum[:, :NLOC],
                                     mybir.ActivationFunctionType.Square,
                                     accum_out=st[:, m + col:m + col + 1])
                nc.vector.tensor_copy(x_sb[:], psum[:])

            def allreduce_stats(st, width, tag):
                ib = dram.tile([128, width], F32, tag=f"arin{tag}")
                ob = dram.tile([128, width], F32, tag=f"arout{tag}")
                g = stats.tile([128, width], F32, tag=f"gst{tag}")
                nc.gpsimd.dma_start(ib[:], st[:])
                if no_ar:
                    nc.gpsimd.dma_start(ob[:], ib[:])
                else:
                    nc.gpsimd.collective_compute(
                        "AllReduce", mybir.AluOpType.add,
                        replica_groups=[list(range(N_CORES))],
                        ins=[ib.opt()], outs=[ob.opt()],
                    )
                nc.gpsimd.dma_start(g[:], ob[:])
                return g

            # ------------------------------------------------------- layer 1
            # stats chunk A: m-tiles [0, sa1); chunk B: [sa1, MT)
            SB1 = MT - sa1
            stats1a = stats.tile([128, 2 * sa1], F32, tag="st1a")
            stats1b = None
            if SB1:
                stats1b = stats.tile([128, 2 * SB1], F32, tag="st1b")
            rois_sb = [None] * len(SLABS)
            x1_sb = []
            g1a = g1b = None
            for mt in range(MT):
                psum = ppmm.tile([128, NPAD], F32, tag="mm")
                for s, (off, w) in enumerate(zip(SLABOFF, SLABS)):
                    if mt == 0:
                        # just-in-time resident rois chunk load
                        rt = consts.tile([128, w, KI, NPAD], F16, tag=f"rois{s}")
                        rois_eng = nc.gpsimd if rois_gp else nc.sync
                        rois_eng.dma_start(
                            rt[:], roisT_d[off:off + w]
                            .rearrange("a p b n -> p a b n"))
                        rois_sb[s] = rt
                    w1t = w1pool.tile([128, w, KI, 128], F16, tag="w1t")
                    nc.sync.dma_start(
                        w1t[:],
                        w1_d[mt, off:off + w].rearrange("a p b f -> p a b f"))
                    for j in range(w):
                        for ki in range(KI):
                            kt = (off + j) * KI + ki
                            nc.tensor.matmul(psum[:], w1t[:, j, ki, :],
                                             rois_sb[s][:, j, ki, :],
                                             start=(kt == 0), stop=(kt == KT1 - 1))
                xt = acts.tile([128, NPAD], F32, tag=f"x1_{mt}")
                if mt < sa1:
                    evict_with_stats(psum, xt, stats1a, mt, sa1)
                else:
                    evict_with_stats(psum, xt, stats1b, mt - sa1, SB1)
                x1_sb.append(xt)
                if mt == sa1 - 1:
                    g1a = allreduce_stats(stats1a, 2 * sa1, "1a")
                if SB1 and mt == MT - 1:
                    g1b = allreduce_stats(stats1b, 2 * SB1, "1b")

            # later-stage weights: emitted after mm1 so their DMAs queue
            # behind the w1 stream (they are needed ~100us in)
            w2_sb = consts.tile([128, MT, 4, 2, 128], F16, tag="w2")
            nc.sync.dma_start(w2_sb[:], w2_d.rearrange("m a p b f -> p m a b f"))
            wh_sb = consts.tile([128, 4, 2, NH], F16, tag="wh")
            nc.sync.dma_start(wh_sb[:], wh_d.rearrange("a p b n -> p a b n"))
            bias_sb = consts.tile([128, NH], F32, tag="biasb")
            nc.sync.dma_start(bias_sb[:], bias_d)

            a1a, c1a = bn_coeffs(g1a, sa1, bn_sb[:, 0:sa1], bn_sb[:, 8:8 + sa1], "1a")
            a1b = c1b = None
            if SB1:
                a1b, c1b = bn_coeffs(g1b, SB1, bn_sb[:, sa1:MT], bn_sb[:, 8 + sa1:16], "1b")

            x1n_sb = []
            for mt in range(MT):
                a_, c_ = (a1a, c1a) if mt < sa1 else (a1b, c1b)
                i = mt if mt < sa1 else mt - sa1
                xn = acts.tile([128, NPAD], F16, tag=f"x1n_{mt}")
                nc.scalar.activation(xn[:], x1_sb[mt][:],
                                     mybir.ActivationFunctionType.Relu,
                                     bias=c_[:, i:i + 1], scale=a_[:, i:i + 1])
                x1n_sb.append(xn)

            def keep_warm(n_pairs, tag):
                # Paced dummy matmuls that keep the PE HAM busy across an
                # AllReduce wait (PE re-throttles to half clock after ~3.4us
                # idle). Each MM->DVE-copy pair self-paces via the bank WAR.
                if not n_pairs:
                    return
                wp = ppmm.tile([128, 128], F32, tag="mm")
                wsc = scr.tile([128, 128], F32, tag=f"wsc{tag}")
                for i in range(n_pairs):
                    nc.tensor.matmul(wp[:], w2_sb[:, 0, 0, 0, :],
                                     w2_sb[:, 0, 0, 0, :], start=True, stop=True)
                    nc.vector.tensor_copy(wsc[:], wp[:])

            keep_warm(warm, "bn1")
            # ------------------------------------------------------- layer 2
            # kt-outer over psum groups of (5, 3) banks — one "mm" slot stays
            # free so the next pipeline iteration's mm1 can proceed while this
            # iteration waits on stats AllReduces. Stats chunk A covers
            # m-tiles [0, sg2), B the rest (sg2=8: single AllReduce).
            sb2 = MT - sg2
            stats2a = stats.tile([128, 2 * sg2], F32, tag="st2a")
            stats2b = None
            if sb2:
                stats2b = stats.tile([128, 2 * sb2], F32, tag="st2b")
            x2_sb = [None] * MT
            g2 = [None, None]
            for grp, mts in enumerate((range(0, 5), range(5, MT))):
                psums = {}
                for m in mts:
                    ps2 = ppmm.tile([128, NPAD], F32, tag="mm")
                    psums[m] = ps2
                for kt in range(MT):
                    for m in mts:
                        nc.tensor.matmul(psums[m][:], w2_sb[:, m, kt // 2, kt % 2, :],
                                         x1n_sb[kt][:],
                                         start=(kt == 0), stop=(kt == MT - 1))
                for m in mts:
                    st, col, w = ((stats2a, m, sg2) if m < sg2 else
                                  (stats2b, m - sg2, sb2))
                    xt = acts.tile([128, NPAD], F32, tag=f"x2_{m}")
                    evict_with_stats(psums[m], xt, st, col, w)
                    x2_sb[m] = xt
                    if m == sg2 - 1:
                        g2[0] = allreduce_stats(stats2a, 2 * sg2, "2a")
                    if sb2 and m == MT - 1:
                        g2[1] = allreduce_stats(stats2b, 2 * sb2, "2b")

            a2a, c2a = bn_coeffs(g2[0], sg2, bn_sb[:, 16:16 + sg2],
                                 bn_sb[:, 24:24 + sg2], "2a")
            a2b = c2b = None
            if sb2:
                a2b, c2b = bn_coeffs(g2[1], sb2, bn_sb[:, 16 + sg2:24],
                                     bn_sb[:, 24 + sg2:32], "2b")

            x2n_sb = []
            for mt in range(MT):
                a_, c_ = (a2a, c2a) if mt < sg2 else (a2b, c2b)
                i = mt if mt < sg2 else mt - sg2
                xn = acts.tile([128, NPAD], F16, tag=f"x2n_{mt}")
                nc.scalar.activation(xn[:], x2_sb[mt][:],
                                     mybir.ActivationFunctionType.Relu,
                                     bias=c_[:, i:i + 1], scale=a_[:, i:i + 1])
                x2n_sb.append(xn)

            keep_warm(warm, "bn2")
            # --------------------------------------------------------- heads
            ph_t = []
            for rt in range(2):
                m = 128 if rt == 0 else NLOC - 128
                ph = pphd.tile([128, NH], F32, tag=f"hd{rt}")
                ph_t.append((ph, m))
            for kt in range(MT):
                for rt in range(2):
                    ph, m = ph_t[rt]
                    nc.tensor.matmul(ph[:m, :], x2n_sb[kt][:, rt * 128:rt * 128 + m],
                                     wh_sb[:, kt // 2, kt % 2, :],
                                     start=(kt == 0), stop=(kt == MT - 1))
            for rt in range(2):
                ph, m = ph_t[rt]
                hb = scr.tile([128, NH], F32, tag="hb")
                nc.vector.tensor_add(hb[:m, :], ph[:m, :], bias_sb[:m, :])
                nc.sync.dma_start(logits_d[rt * 128:rt * 128 + m, :], hb[:m, 0:NC])
                nc.sync.dma_start(deltas_d[rt * 128:rt * 128 + m, :], hb[:m, NC:NH])

                nmx = scr.tile([128, 1], F32, tag="nmx")
                nc.vector.reduce_max(nmx[:m, :], hb[:m, 0:NC],
                                     axis=mybir.AxisListType.X, negate=True)
                ex = scr.tile([128, NC], F32, tag="ex")
                ssum = scr.tile([128, 1], F32, tag="ssum")
                nc.scalar.activation(ex[:m, :], hb[:m, 0:NC],
                                     mybir.ActivationFunctionType.Exp,
                                     bias=nmx[:m, 0:1], accum_out=ssum[:m, 0:1])
                rs = scr.tile([128, 1], F32, tag="rs")
                nc.vector.reciprocal(rs[:m, :], ssum[:m, :])
                pr = scr.tile([128, NC], F32, tag="pr")
                nc.vector.tensor_scalar_mul(pr[:m, :], ex[:m, :], rs[:m, 0:1])
                nc.sync.dma_start(probs_d[rt * 128:rt * 128 + m, :], pr[:m, :])

    nc.compile()
    return nc


def _prep_inputs(pooled_rois, w1, gamma1, beta1, w2, gamma2, beta2,
                 w_logits, b_logits, w_delta, b_delta):
    """Host-side sharding + layout prep. Returns per-core in_maps."""
    rois = np.ascontiguousarray(pooled_rois.reshape(N, K1)).astype(np.float16)
    rois_pad = np.zeros((N_CORES * NPAD, K1), np.float16)
    for c in range(N_CORES):
        rois_pad[c * NPAD:c * NPAD + NLOC] = rois[c * NLOC:(c + 1) * NLOC]

    w1t = np.ascontiguousarray(
        w1.astype(np.float16).reshape(KT7, KI, 128, MT, 128).transpose(3, 0, 2, 1, 4))
    w2t = np.ascontiguousarray(
        w2.astype(np.float16).reshape(4, 2, 128, MT, 128).transpose(3, 0, 2, 1, 4))
    wh = np.concatenate([w_logits, w_delta], axis=1)  # [1024, 405]
    wht = np.ascontiguousarray(
        wh.astype(np.float16).reshape(4, 2, 128, NH).transpose(0, 2, 1, 3))
    bnp = np.ascontiguousarray(
        np.concatenate([gamma1.reshape(MT, 128).T, beta1.reshape(MT, 128).T,
                        gamma2.reshape(MT, 128).T, beta2.reshape(MT, 128).T],
                       axis=1).astype(np.float32))  # [128, 32]
    biasb = np.ascontiguousarray(
        np.broadcast_to(np.concatenate([b_logits, b_delta]).astype(np.float32),
                        (128, NH)))

    in_maps = []
    for c in range(N_CORES):
        roisT = rois_pad[c * NPAD:(c + 1) * NPAD].T  # [K1, NPAD]
        roisT = np.ascontiguousarray(
            roisT.reshape(KT7, KI, 128, NPAD).transpose(0, 2, 1, 3))
        in_maps.append({
            "roisT": roisT,
            "w1t": w1t, "w2t": w2t, "wht": wht,
            "bnp": bnp, "biasb": biasb,
        })
    return in_maps


def get_nc(iters=1, no_ar=False, sa1=SA1, sg2=4, rois_gp=False, warm=0):
    key = ("nc", iters, no_ar, sa1, sg2, rois_gp, warm)
    if key not in _CACHE:
        _CACHE[key] = _build(iters, no_ar, sa1, sg2, rois_gp, warm)
    return _CACHE[key]


def kernel(pooled_rois, w1, b1, gamma1, beta1, w2, b2, gamma2, beta2,
           w_logits, b_logits, w_delta, b_delta):
    # b1/b2 cancel exactly in training-mode batchnorm; unused by design.
    in_maps = _prep_inputs(np.asarray(pooled_rois, np.float32),
                           np.asarray(w1, np.float32),
                           np.asarray(gamma1, np.float32),
                           np.asarray(beta1, np.float32),
                           np.asarray(w2, np.float32),
                           np.asarray(gamma2, np.float32),
                           np.asarray(beta2, np.float32),
                           np.asarray(w_logits, np.float32),
                           np.asarray(b_logits, np.float32),
                           np.asarray(w_delta, np.float32),
                           np.asarray(b_delta, np.float32))
    nc = get_nc(sa1=8, sg2=8, rois_gp=True, warm=8)
    res = bass_utils.run_bass_kernel_spmd(
        nc, in_maps, core_ids=list(range(N_CORES)), trace=False)
    logits = np.concatenate([res.results[c]["logits"] for c in range(N_CORES)])
    probs = np.concatenate([res.results[c]["probs"] for c in range(N_CORES)])
    deltas = np.concatenate([res.results[c]["deltas"] for c in range(N_CORES)])
    return logits, probs, deltas.reshape(N, NC, 4)


# revision 26
# speedup vs baseline: 1.5136x; 1.5136x over previous
"""BBoxHead (nms_detection) Trainium2 Bass kernel, 8-core SPMD.

Strategy
--------
Data-parallel over the RoI axis: 2000 RoIs -> 250/core (padded to 256).
Activations are kept transposed as [feature, roi] on-chip so that:
  * every weight matrix loads from DRAM in its natural [K, M] layout
    (no transposes anywhere),
  * BatchNorm (training mode, batch stats over ALL 2000 RoIs) reduces along
    the free axis; the cross-core part of the stats is a tiny f32 AllReduce
    (sum and sum-of-squares per feature), split into two chunks per layer so
    the collective latency overlaps the surrounding matmuls,
  * the class/delta heads come out as [roi, class] directly (lhsT = x2n),
    so softmax also reduces along the free axis.
Matmul operands are fp16 (full PE rate, fp32 PSUM accumulation); everything
else (BN, softmax, biases, outputs) is fp32.  The conv biases b1/b2 cancel
exactly in training-mode BN and are skipped.  DMAs are emitted in first-use
order (rois/w1 slabs just-in-time, later-stage weights after mm1) so the PE
never waits behind loads it does not need yet.

Host-side prep reorders weights so each SBUF partition receives >=512B
contiguous chunks (two k-rows interleaved per partition).
"""

import numpy as np

import concourse.bass as bass
import concourse.mybir as mybir
import concourse.tile as tile
from concourse import bacc
from concourse import bass_utils

# ---------------------------------------------------------------- constants
N_CORES = 8
N = 2000                      # total RoIs
NLOC = N // N_CORES           # real RoIs per core (250)
NPAD = 256                    # padded RoI columns per core
K1 = 12544                    # 7*7*256 contraction of conv1
HID = 1024
NC = 81
ND = NC * 4                   # 324
NH = NC + ND                  # 405 (heads fused side by side)
EPS = 1e-3
KT1 = K1 // 128               # 98 k-tiles
KI = 7                        # k-tiles interleaved per partition chunk
KT7 = KT1 // KI               # 14 interleaved k-group tiles
# k-group tiles per w1/rois DMA slab (uniform; the HWDGE ring is FIFO so many
# small DMAs cost more than a faster start is worth)
SLABS = [2] * 7
assert sum(SLABS) == KT7
SLABOFF = [sum(SLABS[:i]) for i in range(len(SLABS))]
MT = HID // 128               # 8 feature tiles
SA1 = 7                       # BN1 stats chunk A covers m-tiles [0, SA1)
F16 = mybir.dt.float16
F32 = mybir.dt.float32

_CACHE = {}


def _build(iters=1, no_ar=False, sa1=SA1, sg2=4, rois_gp=False, warm=0,
           w1_alt=False, w1_bufs=4):
    """Build the NEFF. iters>1 repeats the pipeline (timing harness);
    no_ar=True replaces the stats AllReduce with a local copy (timing only —
    wrong numerics, same structure). sa1/sg2 control the BN1/BN2 stats
    chunking (8 = single AllReduce per layer)."""
    nc = bacc.Bacc("TRN2", target_bir_lowering=False, debug=False,
                   enable_asserts=False, num_devices=N_CORES)

    # ------------------------------------------------ DRAM I/O declarations
    roisT_d = nc.dram_tensor("roisT", [KT7, 128, KI, NPAD], F16, kind="ExternalInput").ap()
    w1_d = nc.dram_tensor("w1t", [MT, KT7, 128, KI, 128], F16, kind="ExternalInput").ap()
    w2_d = nc.dram_tensor("w2t", [MT, 4, 128, 2, 128], F16, kind="ExternalInput").ap()
    wh_d = nc.dram_tensor("wht", [4, 128, 2, NH], F16, kind="ExternalInput").ap()
    bn_d = nc.dram_tensor("bnp", [128, 32], F32, kind="ExternalInput").ap()
    bias_d = nc.dram_tensor("biasb", [128, NH], F32, kind="ExternalInput").ap()

    logits_d = nc.dram_tensor("logits", [NLOC, NC], F32, kind="ExternalOutput").ap()
    probs_d = nc.dram_tensor("probs", [NLOC, NC], F32, kind="ExternalOutput").ap()
    deltas_d = nc.dram_tensor("deltas", [NLOC, ND], F32, kind="ExternalOutput").ap()

    with tile.TileContext(nc) as tc:
        with (
            tc.tile_pool(name="consts", bufs=1) as consts,
            tc.tile_pool(name="w1pool", bufs=w1_bufs) as w1pool,
            tc.tile_pool(name="acts", bufs=1) as acts,
            tc.tile_pool(name="scr", bufs=2) as scr,
            tc.tile_pool(name="stats", bufs=1) as stats,
            tc.tile_pool(name="ppmm", bufs=6, space="PSUM") as ppmm,
            tc.tile_pool(name="pphd", bufs=1, space="PSUM") as pphd,
            tc.tile_pool(name="dram", bufs=1, space="DRAM") as dram,
        ):
          for _it in range(iters):
            # ---------------- tiny consts needed early (BN params, eps)
            bn_sb = consts.tile([128, 32], F32, tag="bn")
            nc.sync.dma_start(bn_sb[:], bn_d)
            eps_sb = consts.tile([128, 1], F32, tag="eps")
            nc.vector.memset(eps_sb[:], EPS)

            # --------------------------------------------------- helpers
            def bn_coeffs(gstats, m, gamma, beta, tag):
                """a = gamma*rsqrt(var+eps); c = beta - mu*a  (all [128, m]).
                gstats: [128, 2m] (sums | sumsqs) of x over the full batch."""
                mu = stats.tile([128, m], F32, tag=f"mu{tag}")
                ex2 = stats.tile([128, m], F32, tag=f"ex2{tag}")
                mu2 = stats.tile([128, m], F32, tag=f"mu2{tag}")
                var = stats.tile([128, m], F32, tag=f"var{tag}")
                std = stats.tile([128, m], F32, tag=f"std{tag}")
                rinv = stats.tile([128, m], F32, tag=f"rinv{tag}")
                a = stats.tile([128, m], F32, tag=f"a{tag}")
                ac = stats.tile([128, m], F32, tag=f"ac{tag}")
                c = stats.tile([128, m], F32, tag=f"c{tag}")
                nc.vector.tensor_scalar_mul(mu[:], gstats[:, 0:m], 1.0 / N)
                nc.vector.tensor_scalar_mul(ex2[:], gstats[:, m:2 * m], 1.0 / N)
                nc.vector.tensor_mul(mu2[:], mu[:], mu[:])
                nc.vector.tensor_sub(var[:], ex2[:], mu2[:])
                nc.scalar.activation(std[:], var[:], mybir.ActivationFunctionType.Sqrt,
                                     bias=eps_sb[:, 0:1])
                nc.vector.reciprocal(rinv[:], std[:])
                nc.vector.tensor_mul(a[:], gamma, rinv[:])
                nc.vector.tensor_mul(ac[:], mu[:], a[:])
                nc.vector.tensor_sub(c[:], beta, ac[:])
                return a, c

            def evict_with_stats(psum, x_sb, st, col, m=MT):
                """psum [128,NPAD] f32 -> x_sb; sum/sumsq of first NLOC cols
                into st columns col and m+col."""
                nc.vector.reduce_sum(st[:, col:col + 1], psum[:, :NLOC],
                                     axis=mybir.AxisListType.X)
                sq = scr.tile([128, NLOC], F32, tag="sq")
                nc.scalar.activation(sq[:], psum[:, :NLOC],
                                     mybir.ActivationFunctionType.Square,
                                     accum_out=st[:, m + col:m + col + 1])
                nc.vector.tensor_copy(x_sb[:], psum[:])

            def allreduce_stats(st, width, tag):
                ib = dram.tile([128, width], F32, tag=f"arin{tag}")
                ob = dram.tile([128, width], F32, tag=f"arout{tag}")
                g = stats.tile([128, width], F32, tag=f"gst{tag}")
                nc.gpsimd.dma_start(ib[:], st[:])
                if no_ar:
                    nc.gpsimd.dma_start(ob[:], ib[:])
                else:
                    nc.gpsimd.collective_compute(
                        "AllReduce", mybir.AluOpType.add,
                        replica_groups=[list(range(N_CORES))],
                        ins=[ib.opt()], outs=[ob.opt()],
                    )
                nc.gpsimd.dma_start(g[:], ob[:])
                return g

            # ------------------------------------------------------- layer 1
            # stats chunk A: m-tiles [0, sa1); chunk B: [sa1, MT)
            SB1 = MT - sa1
            stats1a = stats.tile([128, 2 * sa1], F32, tag="st1a")
            stats1b = None
            if SB1:
                stats1b = stats.tile([128, 2 * SB1], F32, tag="st1b")
            rois_sb = [None] * len(SLABS)
            x1_sb = []
            g1a = g1b = None
            for mt in range(MT):
                psum = ppmm.tile([128, NPAD], F32, tag="mm")
                for s, (off, w) in enumerate(zip(SLABOFF, SLABS)):
                    if mt == 0:
                        # just-in-time resident rois chunk load
                        rt = consts.tile([128, w, KI, NPAD], F16, tag=f"rois{s}")
                        rois_eng = nc.gpsimd if rois_gp else nc.sync
                        rois_eng.dma_start(
                            rt[:], roisT_d[off:off + w]
                            .rearrange("a p b n -> p a b n"))
                        rois_sb[s] = rt
                    w1t = w1pool.tile([128, w, KI, 128], F16, tag="w1t")
                    w1_eng = nc.gpsimd if (w1_alt and s % 2) else nc.sync
                    w1_eng.dma_start(
                        w1t[:],
                        w1_d[mt, off:off + w].rearrange("a p b f -> p a b f"))
                    for j in range(w):
                        for ki in range(KI):
                            kt = (off + j) * KI + ki
                            nc.tensor.matmul(psum[:], w1t[:, j, ki, :],
                                             rois_sb[s][:, j, ki, :],
                                             start=(kt == 0), stop=(kt == KT1 - 1))
                xt = acts.tile([128, NPAD], F32, tag=f"x1_{mt}")
                if mt < sa1:
                    evict_with_stats(psum, xt, stats1a, mt, sa1)
                else:
                    evict_with_stats(psum, xt, stats1b, mt - sa1, SB1)
                x1_sb.append(xt)
                if mt == sa1 - 1:
                    g1a = allreduce_stats(stats1a, 2 * sa1, "1a")
                if SB1 and mt == MT - 1:
                    g1b = allreduce_stats(stats1b, 2 * SB1, "1b")

            # later-stage weights: emitted after mm1 so their DMAs queue
            # behind the w1 stream (they are needed ~100us in)
            w2_sb = consts.tile([128, MT, 4, 2, 128], F16, tag="w2")
            nc.sync.dma_start(w2_sb[:], w2_d.rearrange("m a p b f -> p m a b f"))
            wh_sb = consts.tile([128, 4, 2, NH], F16, tag="wh")
            nc.sync.dma_start(wh_sb[:], wh_d.rearrange("a p b n -> p a b n"))
            bias_sb = consts.tile([128, NH], F32, tag="biasb")
            nc.sync.dma_start(bias_sb[:], bias_d)

            a1a, c1a = bn_coeffs(g1a, sa1, bn_sb[:, 0:sa1], bn_sb[:, 8:8 + sa1], "1a")
            a1b = c1b = None
            if SB1:
                a1b, c1b = bn_coeffs(g1b, SB1, bn_sb[:, sa1:MT], bn_sb[:, 8 + sa1:16], "1b")

            x1n_sb = []
            for mt in range(MT):
                a_, c_ = (a1a, c1a) if mt < sa1 else (a1b, c1b)
                i = mt if mt < sa1 else mt - sa1
                xn = acts.tile([128, NPAD], F16, tag=f"x1n_{mt}")
                nc.scalar.activation(xn[:], x1_sb[mt][:],
                                     mybir.ActivationFunctionType.Relu,
                                     bias=c_[:, i:i + 1], scale=a_[:, i:i + 1])
                x1n_sb.append(xn)

            def keep_warm(n_pairs, tag):
                # Paced dummy matmuls that keep the PE HAM busy across an
                # AllReduce wait (PE re-throttles to half clock after ~3.4us
                # idle). Each MM->DVE-copy pair self-paces via the bank WAR.
                if not n_pairs:
                    return
                wp = ppmm.tile([128, 128], F32, tag="mm")
                wsc = scr.tile([128, 128], F32, tag=f"wsc{tag}")
                for i in range(n_pairs):
                    nc.tensor.matmul(wp[:], w2_sb[:, 0, 0, 0, :],
                                     w2_sb[:, 0, 0, 0, :], start=True, stop=True)
                    nc.vector.tensor_copy(wsc[:], wp[:])

            keep_warm(warm, "bn1")
            # ------------------------------------------------------- layer 2
            # kt-outer over psum groups of (5, 3) banks — one "mm" slot stays
            # free so the next pipeline iteration's mm1 can proceed while this
            # iteration waits on stats AllReduces. Stats chunk A covers
            # m-tiles [0, sg2), B the rest (sg2=8: single AllReduce).
            sb2 = MT - sg2
            stats2a = stats.tile([128, 2 * sg2], F32, tag="st2a")
            stats2b = None
            if sb2:
                stats2b = stats.tile([128, 2 * sb2], F32, tag="st2b")
            x2_sb = [None] * MT
            g2 = [None, None]
            for grp, mts in enumerate((range(0, 5), range(5, MT))):
                psums = {}
                for m in mts:
                    ps2 = ppmm.tile([128, NPAD], F32, tag="mm")
                    psums[m] = ps2
                for kt in range(MT):
                    for m in mts:
                        nc.tensor.matmul(psums[m][:], w2_sb[:, m, kt // 2, kt % 2, :],
                                         x1n_sb[kt][:],
                                         start=(kt == 0), stop=(kt == MT - 1))
                for m in mts:
                    st, col, w = ((stats2a, m, sg2) if m < sg2 else
                                  (stats2b, m - sg2, sb2))
                    xt = acts.tile([128, NPAD], F32, tag=f"x2_{m}")
                    evict_with_stats(psums[m], xt, st, col, w)
                    x2_sb[m] = xt
                    if m == sg2 - 1:
                        g2[0] = allreduce_stats(stats2a, 2 * sg2, "2a")
                    if sb2 and m == MT - 1:
                        g2[1] = allreduce_stats(stats2b, 2 * sb2, "2b")

            a2a, c2a = bn_coeffs(g2[0], sg2, bn_sb[:, 16:16 + sg2],
                                 bn_sb[:, 24:24 + sg2], "2a")
            a2b = c2b = None
            if sb2:
                a2b, c2b = bn_coeffs(g2[1], sb2, bn_sb[:, 16 + sg2:24],
                                     bn_sb[:, 24 + sg2:32], "2b")

            x2n_sb = []
            for mt in range(MT):
                a_, c_ = (a2a, c2a) if mt < sg2 else (a2b, c2b)
                i = mt if mt < sg2 else mt - sg2
                xn = acts.tile([128, NPAD], F16, tag=f"x2n_{mt}")
                nc.scalar.activation(xn[:], x2_sb[mt][:],
                                     mybir.ActivationFunctionType.Relu,
                                     bias=c_[:, i:i + 1], scale=a_[:, i:i + 1])
                x2n_sb.append(xn)

            keep_warm(warm, "bn2")
            # --------------------------------------------------------- heads
            ph_t = []
            for rt in range(2):
                m = 128 if rt == 0 else NLOC - 128
                ph = pphd.tile([128, NH], F32, tag=f"hd{rt}")
                ph_t.append((ph, m))
            for kt in range(MT):
                for rt in range(2):
                    ph, m = ph_t[rt]
                    nc.tensor.matmul(ph[:m, :], x2n_sb[kt][:, rt * 128:rt * 128 + m],
                                     wh_sb[:, kt // 2, kt % 2, :],
                                     start=(kt == 0), stop=(kt == MT - 1))
            for rt in range(2):
                ph, m = ph_t[rt]
                hb = scr.tile([128, NH], F32, tag="hb")
                nc.vector.tensor_add(hb[:m, :], ph[:m, :], bias_sb[:m, :])
                nc.sync.dma_start(logits_d[rt * 128:rt * 128 + m, :], hb[:m, 0:NC])
                nc.sync.dma_start(deltas_d[rt * 128:rt * 128 + m, :], hb[:m, NC:NH])

                nmx = scr.tile([128, 1], F32, tag="nmx")
                nc.vector.reduce_max(nmx[:m, :], hb[:m, 0:NC],
                                     axis=mybir.AxisListType.X, negate=True)
                ex = scr.tile([128, NC], F32, tag="ex")
                ssum = scr.tile([128, 1], F32, tag="ssum")
                nc.scalar.activation(ex[:m, :], hb[:m, 0:NC],
                                     mybir.ActivationFunctionType.Exp,
                                     bias=nmx[:m, 0:1], accum_out=ssum[:m, 0:1])
                rs = scr.tile([128, 1], F32, tag="rs")
                nc.vector.reciprocal(rs[:m, :], ssum[:m, :])
                pr = scr.tile([128, NC], F32, tag="pr")
                nc.vector.tensor_scalar_mul(pr[:m, :], ex[:m, :], rs[:m, 0:1])
                nc.sync.dma_start(probs_d[rt * 128:rt * 128 + m, :], pr[:m, :])

    nc.compile()
    return nc


def _prep_inputs(pooled_rois, w1, gamma1, beta1, w2, gamma2, beta2,
                 w_logits, b_logits, w_delta, b_delta):
    """Host-side sharding + layout prep. Returns per-core in_maps."""
    rois = np.ascontiguousarray(pooled_rois.reshape(N, K1)).astype(np.float16)
    rois_pad = np.zeros((N_CORES * NPAD, K1), np.float16)
    for c in range(N_CORES):
        rois_pad[c * NPAD:c * NPAD + NLOC] = rois[c * NLOC:(c + 1) * NLOC]

    w1t = np.ascontiguousarray(
        w1.astype(np.float16).reshape(KT7, KI, 128, MT, 128).transpose(3, 0, 2, 1, 4))
    w2t = np.ascontiguousarray(
        w2.astype(np.float16).reshape(4, 2, 128, MT, 128).transpose(3, 0, 2, 1, 4))
    wh = np.concatenate([w_logits, w_delta], axis=1)  # [1024, 405]
    wht = np.ascontiguousarray(
        wh.astype(np.float16).reshape(4, 2, 128, NH).transpose(0, 2, 1, 3))
    bnp = np.ascontiguousarray(
        np.concatenate([gamma1.reshape(MT, 128).T, beta1.reshape(MT, 128).T,
                        gamma2.reshape(MT, 128).T, beta2.reshape(MT, 128).T],
                       axis=1).astype(np.float32))  # [128, 32]
    biasb = np.ascontiguousarray(
        np.broadcast_to(np.concatenate([b_logits, b_delta]).astype(np.float32),
                        (128, NH)))

    in_maps = []
    for c in range(N_CORES):
        roisT = rois_pad[c * NPAD:(c + 1) * NPAD].T  # [K1, NPAD]
        roisT = np.ascontiguousarray(
            roisT.reshape(KT7, KI, 128, NPAD).transpose(0, 2, 1, 3))
        in_maps.append({
            "roisT": roisT,
            "w1t": w1t, "w2t": w2t, "wht": wht,
            "bnp": bnp, "biasb": biasb,
        })
    return in_maps


def get_nc(iters=1, no_ar=False, sa1=SA1, sg2=4, rois_gp=False, warm=0,
           w1_alt=False, w1_bufs=4):
    key = ("nc", iters, no_ar, sa1, sg2, rois_gp, warm, w1_alt, w1_bufs)
    if key not in _CACHE:
        _CACHE[key] = _build(iters, no_ar, sa1, sg2, rois_gp, warm, w1_alt, w1_bufs)
    return _CACHE[key]


def kernel(pooled_rois, w1, b1, gamma1, beta1, w2, b2, gamma2, beta2,
           w_logits, b_logits, w_delta, b_delta):
    # b1/b2 cancel exactly in training-mode batchnorm; unused by design.
    in_maps = _prep_inputs(np.asarray(pooled_rois, np.float32),
                           np.asarray(w1, np.float32),
                           np.asarray(gamma1, np.float32),
                           np.asarray(beta1, np.float32),
                           np.asarray(w2, np.float32),
                           np.asarray(gamma2, np.float32),
                           np.asarray(beta2, np.float32),
                           np.asarray(w_logits, np.float32),
                           np.asarray(b_logits, np.float32),
                           np.asarray(w_delta, np.float32),
                           np.asarray(b_delta, np.float32))
    nc = get_nc(sa1=8, sg2=8, rois_gp=True, warm=8)
    res = bass_utils.run_bass_kernel_spmd(
        nc, in_maps, core_ids=list(range(N_CORES)), trace=False)
    logits = np.concatenate([res.results[c]["logits"] for c in range(N_CORES)])
    probs = np.concatenate([res.results[c]["probs"] for c in range(N_CORES)])
    deltas = np.concatenate([res.results[c]["deltas"] for c in range(N_CORES)])
    return logits, probs, deltas.reshape(N, NC, 4)
